# Initial kernel scaffold
#
"""Multi-head graph attention (GAT) Trainium2 kernel, 8-core SPMD.

Problem: h[4096,256], adj[4096,4096] bool, w[4,256,64], a_src/a_dst[4,64,1],
bias[64] -> out[4096,4,64]:
    h_prime = h @ w[k]                       per head
    s[i,j]  = src[i] + dst[j]                (rank-1!)
    scores  = leaky_relu(s, 0.2), masked by adj, softmax over j
    out     = attn @ h_prime + bias

Sharding: 8 cores = 2 head-groups x 4 row-blocks. Core c computes heads
[2*(c%2), 2*(c%2)+1] for output rows [1024*(c//2), 1024*(c//2)+1024).

Key algebra (all on-device, per head):
    exp(leaky(s)) = max(e^s, e^{0.2 s}) = e^{0.2 s} * max(e^{0.8 s}, 1)
    e^{0.2 s} = e^{0.2 src_i} * e^{0.2 dst_j};  e^{0.8 s} factors likewise.
The per-column factor e^{0.2 src_i} cancels in the softmax normalization,
so the unnormalized weights reduce to
    P'[j,i] = e^{0.2 dst_j - 20} * adj[i,j] * max(e^{0.8 src_i} * e^{0.8 dst_j}, 1)
which is exactly two VectorE ops per tile: a dual-op tensor_scalar
(mult + max-with-1) on a replicated e^{0.8 src} tile, and one fused
scalar_tensor_tensor (x per-partition scalar, x adj mask). The softmax
denominator comes for free from a ones-column appended to h_prime in the
final matmul; the epilogue divides by it (the dropped factors cancel).
"""

import sys

sys.path.insert(0, "/opt/trn_rl_repo")

import numpy as np
import ml_dtypes

N = 4096          # nodes
F = 256           # f_in
O = 64            # f_out
NHEAD = 4
NCORES = 8
NH = 2            # heads per core
NI = 1024         # output rows per core
NCJ = N // 128    # 32 j-chunks
NSEG = NI // 512  # 2 segments of 512 in the i (free) dim
NSUB = NI // 128  # 8 i-subtiles of 128
CB = 20.0         # shift inside e^{0.2 dst - CB} to keep bf16 range safe

_CACHE = {}


def _build():
    import concourse.bass as bass
    import concourse.bacc as bacc
    import concourse.mybir as mybir
    import concourse.tile as tile
    from concourse.bass import ts

    from concourse.masks import make_identity

    f32 = mybir.dt.float32
    bf16 = mybir.dt.bfloat16
    Alu = mybir.AluOpType
    Act = mybir.ActivationFunctionType

    nc = bacc.Bacc()
    hT_d = nc.declare_dram_parameter("hT", [F, N], bf16, isOutput=False)
    hTi_d = nc.declare_dram_parameter("hTi", [F, NI], bf16, isOutput=False)
    adjT_d = nc.declare_dram_parameter("adjT", [8 * 128, 4 * NI], bf16, isOutput=False)
    wr_d = nc.declare_dram_parameter("wr", [F, NH * O], bf16, isOutput=False)
    wTr_d = nc.declare_dram_parameter("wTr", [O, NH * F], bf16, isOutput=False)
    avec_d = nc.declare_dram_parameter("avec", [O, 2 * NH], bf16, isOutput=False)
    bias_d = nc.declare_dram_parameter("biasd", [1, O], f32, isOutput=False)
    out_d = nc.declare_dram_parameter("out", [NH, 128, NSUB * O], f32, isOutput=True)

    with tile.TileContext(nc) as tc:
        with (
            tc.tile_pool(name="sb", bufs=1) as sb,
            tc.tile_pool(name="sbr", bufs=2) as sbr,
            tc.tile_pool(name="sbo", bufs=3) as sbo,
            tc.tile_pool(name="pw", bufs=4, space="PSUM") as pw,
            tc.tile_pool(name="pacc", bufs=1, space="PSUM") as pacc,
        ):
            # ---- static SBUF tensors ----
            hT_sb = sb.tile([128, 2, N], bf16, name="hT_sb")
            hTi_sb = sb.tile([128, 2, NI], bf16, name="hTi_sb")
            adjT_sb = sb.tile([128, NCJ, NI], bf16, name="adjT_sb")
            wTr_sb = sb.tile([O, NH, F], bf16, name="wTr_sb")
            avec_sb = sb.tile([O, 2 * NH], bf16, name="avec_sb")
            bias_sb = sb.tile([1, O], f32, name="bias_sb")
            wall_sb = sb.tile([128, 2, NH * O + NH], bf16, name="wall_sb")
            vsrc_sb = sb.tile([128, 2, NH], bf16, name="vsrc_sb")
            ones_row = sb.tile([1, 128], bf16, name="ones_row")
            esrc3_rep = sb.tile([128, NH, NI], bf16, name="esrc3_rep")
            esrc3_row = sb.tile([1, NH, NI], bf16, name="esrc3_row")
            g_sb = sb.tile([128, NH, NCJ, O + 1], bf16, name="g_sb")
            edst3_sb = sb.tile([128, NCJ, NH], f32, name="edst3_sb")
            edst2_sb = sb.tile([128, NCJ, NH], f32, name="edst2_sb")
            bias_rep = sb.tile([128, O], f32, name="bias_rep")
            ostage = sb.tile([128, NH, NSUB, O], f32, name="ostage")
            negcb = sb.tile([128, 1], f32, name="negcb")
            nc.vector.memset(negcb[:, :], -CB)
            zerob = sb.tile([128, 1], f32, name="zerob")
            nc.vector.memset(zerob[:, :], 0.0)
            ident = sb.tile([128, 128], f32, name="ident")
            make_identity(nc, ident[:, :])

            # ---- DMA in ----  (small control tensors FIRST so the prologue
            # matmul chain can start while the bulk hT/adjT loads stream in)
            nc.sync.dma_start(
                hTi_sb, hTi_d[:, :].rearrange("(fc p) i -> p fc i", p=128)
            )
            wTr_r = wTr_d[:, :].rearrange("o (h f) -> o h f", h=NH)
            for h in range(NH):
                nc.sync.dma_start(wTr_sb[:, h, :], wTr_r[:, h, :])
            nc.sync.dma_start(avec_sb, avec_d[:, :])
            nc.sync.dma_start(bias_sb, bias_d[:, :])
            nc.sync.dma_start(
                wall_sb[:, :, 0 : NH * O],
                wr_d[:, :].rearrange("(fc p) m -> p fc m", p=128),
            )
            # adjT is host-pre-tiled as [8 groups, 128 partitions, 4*NI]:
            # each group DMA is 1 MiB with 8 KiB-contiguous runs per
            # partition (large-descriptor regime, ~340 GB/s)
            adjT_r = adjT_d[:, :].rearrange("(g p) x -> g p x", p=128)
            nc.sync.dma_start(
                adjT_sb[:, 0:4, :].rearrange("p c i -> p (c i)"), adjT_r[0]
            )
            hT_r = hT_d[:, :].rearrange("(fc p) j -> p fc j", p=128)
            nc.sync.dma_start(hT_sb[:, :, 0:512], hT_r[:, :, 0:512])
            nc.sync.dma_start(hT_sb[:, :, 512:N], hT_r[:, :, 512:N])
            for g in range(1, 7, 2):
                nc.sync.dma_start(
                    adjT_sb[:, 4 * g : 4 * g + 8, :].rearrange(
                        "p (g c) i -> p g (c i)", g=2
                    ),
                    adjT_r[g : g + 2].rearrange("g p x -> p g x"),
                )
            nc.sync.dma_start(
                adjT_sb[:, 28:32, :].rearrange("p c i -> p (c i)"), adjT_r[7]
            )

            nc.vector.memset(ones_row[:, :], 1.0)
            ones_f = sb.tile([1, 128], f32, name="ones_f")
            nc.vector.memset(ones_f[:, :], 1.0)

            # ---- v vectors: v[f] = sum_o wT[o,f] * a[o]  (cols: src, dst)
            for h in range(NH):
                for fc in range(2):
                    v_ps = pw.tile([128, 512], f32, name=f"v_ps_{h}_{fc}", tag="scratch")
                    nc.tensor.matmul(
                        v_ps[:, 0:2],
                        lhsT=wTr_sb[:, h, ts(fc, 128)],
                        rhs=avec_sb[:, 2 * h : 2 * h + 2],
                        start=True,
                        stop=True,
                    )
                    nc.scalar.copy(vsrc_sb[:, fc, h : h + 1], v_ps[:, 0:1])
                    nc.scalar.copy(
                        wall_sb[:, fc, NH * O + h : NH * O + h + 1], v_ps[:, 1:2]
                    )

            # ---- src row for this core's i-block, per head, then exp
            for h in range(NH):
                for seg in range(NSEG):
                    sr_ps = pw.tile([128, 512], f32, name=f"sr_ps_{h}_{seg}", tag="scratch")
                    for fc in range(2):
                        nc.tensor.matmul(
                            sr_ps[0:1, :],
                            lhsT=vsrc_sb[:, fc, h : h + 1],
                            rhs=hTi_sb[:, fc, ts(seg, 512)],
                            start=(fc == 0),
                            stop=(fc == 1),
                        )
                    nc.scalar.activation(
                        esrc3_row[:, h, ts(seg, 512)],
                        sr_ps[0:1, :],
                        Act.Exp,
                        scale=0.8,
                        bias=zerob[0:1, :],
                    )

            # ---- replicate e^{0.8 src} across partitions (K=1 ones matmul)
            for h in range(NH):
                for seg in range(NSEG):
                    rep_ps = pw.tile([128, 512], f32, name=f"rep_ps_{h}_{seg}", tag="scratch")
                    nc.tensor.matmul(
                        rep_ps[:, :],
                        lhsT=ones_row[:, :],
                        rhs=esrc3_row[:, h, ts(seg, 512)],
                        start=True,
                        stop=True,
                    )
                    nc.scalar.copy(esrc3_rep[:, h, ts(seg, 512)], rep_ps[:, :])


            # ---- bmm accumulators: psum [65, 512] per (head, i-segment)
            acc = [
                pacc.tile([O + 1, 512], f32, name=f"acc{g}", tag=f"acc{g}")
                for g in range(NH * NSEG)
            ]

            # ---- main loop over j-chunks, with the PE/ACT producer chain
            # (h_prime -> exp scalars -> G) running PIPE chunks ahead of the
            # DVE consumer so the vector engine never stalls on it.
            PIPE = 4

            def hp_block(c):
                # h_prime (2 heads) + dst (2 heads) in one accumulated matmul
                hp_ps = pw.tile([128, 512], f32, name=f"hp_ps_{c}", tag="scratch")[:, 0 : NH * O + NH]
                for fc in range(2):
                    nc.tensor.matmul(
                        hp_ps[:, :],
                        lhsT=hT_sb[:, fc, ts(c, 128)],
                        rhs=wall_sb[:, fc, :],
                        start=(fc == 0),
                        stop=(fc == 1),
                    )
                # exp the dst columns straight out of psum
                nc.scalar.activation(
                    edst3_sb[:, c, :],
                    hp_ps[:, NH * O : NH * O + NH],
                    Act.Exp,
                    scale=0.8,
                    bias=zerob[:, :],
                )
                nc.scalar.activation(
                    edst2_sb[:, c, :],
                    hp_ps[:, NH * O : NH * O + NH],
                    Act.Exp,
                    scale=0.2,
                    bias=negcb[:, :],
                )
                # G = e^{0.2 dst - CB} * [h_prime | 1] in bf16 (the edst2
                # row-factor rides the stationary operand; col O holds
                # edst2 itself, which makes the matmul's last row the
                # softmax denominator)
                for h in range(NH):
                    nc.scalar.activation(
                        g_sb[:, h, c, 0:O],
                        hp_ps[:, ts(h, O)],
                        Act.Copy,
                        scale=edst2_sb[:, c, h : h + 1],
                    )
                    nc.scalar.copy(g_sb[:, h, c, O : O + 1], edst2_sb[:, c, h : h + 1])

            for c in range(PIPE):
                hp_block(c)

            for c in range(NCJ):
                if c + PIPE < NCJ:
                    hp_block(c + PIPE)
                for h in range(NH):
                    r_t = sbr.tile([128, NI], bf16, name=f"r_{h}_{c}", tag=f"R{h}", bufs=4)
                    nc.vector.tensor_scalar(
                        out=r_t[:, :],
                        in0=esrc3_rep[:, h, :],
                        scalar1=edst3_sb[:, c, h : h + 1],
                        scalar2=1.0,
                        op0=Alu.mult,
                        op1=Alu.max,
                    )
                    p_t = sbr.tile([128, NI], bf16, name=f"p_{h}_{c}", tag=f"P{h}", bufs=6)
                    nc.vector.tensor_tensor(
                        out=p_t[:, :],
                        in0=r_t[:, :],
                        in1=adjT_sb[:, c, :],
                        op=Alu.mult,
                    )
                    for seg in range(NSEG):
                        nc.tensor.matmul(
                            acc[h * NSEG + seg][:, :],
                            lhsT=g_sb[:, h, c, :],
                            rhs=p_t[:, ts(seg, 512)],
                            start=(c == 0),
                            stop=(c == NCJ - 1),
                        )

            # ---- epilogue: transpose [65,512] -> [128,65], divide, bias,
            # stage per head, then one big DMA per head
            for h in range(NH):
                for seg in range(NSEG):
                    a_ps = acc[h * NSEG + seg]
                    tr_in = sbo.tile([O + 1, 512], f32, name=f"tr_{h}_{seg}", tag="trin")
                    nc.scalar.copy(tr_in[:, :], a_ps[:, :])
                    for q in range(4):
                        isub = seg * 4 + q
                        tr_ps = pw.tile([128, 512], f32, name=f"trp_{h}_{isub}", tag="scratch")
                        nc.tensor.transpose(
                            tr_ps[:, 0 : O + 1],
                            tr_in[:, ts(q, 128)],
                            ident[0 : O + 1, 0 : O + 1],
                        )
                        rec = sbr.tile([128, 1], f32, name=f"rec_{h}_{isub}", tag="rec")
                        nc.vector.reciprocal(rec[:, :], tr_ps[:, O : O + 1])
                        nc.scalar.activation(
                            ostage[:, h, isub, :],
                            tr_ps[:, 0:O],
                            Act.Copy,
                            scale=rec[:, :],
                        )
                nc.sync.dma_start(
                    out_d[h, :, :], ostage[:, h, :, :].rearrange("p s o -> p (s o)")
                )

    nc.finalize()
    return nc


def _prep_inputs(h, adj, w, a_src, a_dst, bias):
    """Host-side sharding / layout prep (no reference math)."""
    h = np.asarray(h, dtype=np.float32)
    adj = np.asarray(adj)
    w = np.asarray(w, dtype=np.float32)
    a_src = np.asarray(a_src, dtype=np.float32)
    a_dst = np.asarray(a_dst, dtype=np.float32)
    bias = np.asarray(bias, dtype=np.float32)

    hT = np.ascontiguousarray(h.T)                       # [F, N]
    adjT = np.ascontiguousarray(adj.T).astype(ml_dtypes.bfloat16)  # [N, N] 0/1

    in_maps = []
    for c in range(NCORES):
        hb, ib = c % 2, c // 2
        heads = [2 * hb, 2 * hb + 1]
        i0 = NI * ib
        w2 = w[heads]                                    # [2, F, O]
        wr = np.ascontiguousarray(w2.transpose(1, 0, 2).reshape(F, NH * O))
        wTr = np.ascontiguousarray(
            np.concatenate([w2[0].T, w2[1].T], axis=1)   # [O, 2F]
        )
        avec = np.ascontiguousarray(
            np.stack(
                [a_src[heads[0], :, 0], a_dst[heads[0], :, 0],
                 a_src[heads[1], :, 0], a_dst[heads[1], :, 0]],
                axis=1,
            )
        )                                                # [O, 4]
        in_maps.append(
            {
                "hT": hT.astype(ml_dtypes.bfloat16),
                "hTi": np.ascontiguousarray(hT[:, i0 : i0 + NI]).astype(
                    ml_dtypes.bfloat16
                ),
                "adjT": np.ascontiguousarray(
                    adjT[:, i0 : i0 + NI]
                    .reshape(8, 4, 128, NI)
                    .transpose(0, 2, 1, 3)
                    .reshape(8 * 128, 4 * NI)
                ),
                "wr": wr.astype(ml_dtypes.bfloat16),
                "wTr": wTr.astype(ml_dtypes.bfloat16),
                "avec": avec.astype(ml_dtypes.bfloat16),
                "biasd": bias.reshape(1, O),
            }
        )
    return in_maps


def kernel(h, adj, w, a_src, a_dst, bias):
    from concourse.bass_utils import run_bass_kernel_spmd

    if "nc" not in _CACHE:
        _CACHE["nc"] = _build()
    nc = _CACHE["nc"]

    in_maps = _prep_inputs(h, adj, w, a_src, a_dst, bias)
    res = run_bass_kernel_spmd(nc, in_maps, list(range(NCORES))).results

    out = np.empty((N, NHEAD, O), dtype=np.float32)
    for c in range(NCORES):
        hb, ib = c % 2, c // 2
        arr = res[c]["out"]  # [NH, 128, NSUB*O]
        for hh in range(NH):
            blk = (
                arr[hh]
                .reshape(128, NSUB, O)
                .transpose(1, 0, 2)
                .reshape(NI, O)
            )
            out[NI * ib : NI * (ib + 1), 2 * hb + hh, :] = blk
    out += np.asarray(bias, dtype=np.float32).reshape(1, 1, O)
    return out



# revision 1
# speedup vs baseline: 3.0707x; 3.0707x over previous
"""Multi-head graph attention (GAT) Trainium2 kernel, 8-core SPMD.

Problem: h[4096,256], adj[4096,4096] bool, w[4,256,64], a_src/a_dst[4,64,1],
bias[64] -> out[4096,4,64]:
    h_prime = h @ w[k]                       per head
    s[i,j]  = src[i] + dst[j]                (rank-1!)
    scores  = leaky_relu(s, 0.2), masked by adj, softmax over j
    out     = attn @ h_prime + bias

Sharding: 8 cores = 2 head-groups x 4 row-blocks. Core c computes heads
[2*(c%2), 2*(c%2)+1] for output rows [1024*(c//2), 1024*(c//2)+1024).

Key algebra (all on-device, per head):
    exp(leaky(s)) = max(e^s, e^{0.2 s}) = e^{0.2 s} * max(e^{0.8 s}, 1)
    e^{0.2 s} = e^{0.2 src_i} * e^{0.2 dst_j};  e^{0.8 s} factors likewise.
The per-column factor e^{0.2 src_i} cancels in the softmax normalization,
so the unnormalized weights reduce to
    P'[j,i] = e^{0.2 dst_j - 20} * adj[i,j] * max(e^{0.8 src_i} * e^{0.8 dst_j}, 1)
which is exactly two VectorE ops per tile: a dual-op tensor_scalar
(mult + max-with-1) on a replicated e^{0.8 src} tile, and one fused
scalar_tensor_tensor (x per-partition scalar, x adj mask). The softmax
denominator comes for free from a ones-column appended to h_prime in the
final matmul; the epilogue divides by it (the dropped factors cancel).
"""

import sys

sys.path.insert(0, "/opt/trn_rl_repo")

import numpy as np
import ml_dtypes

N = 4096          # nodes
F = 256           # f_in
O = 64            # f_out
NHEAD = 4
NCORES = 8
NH = 2            # heads per core
NI = 1024         # output rows per core
NCJ = N // 128    # 32 j-chunks
NSEG = NI // 512  # 2 segments of 512 in the i (free) dim
NSUB = NI // 128  # 8 i-subtiles of 128
CB = 20.0         # shift inside e^{0.2 dst - CB} to keep bf16 range safe

_CACHE = {}


def _build():
    import concourse.bass as bass
    import concourse.bacc as bacc
    import concourse.mybir as mybir
    import concourse.tile as tile
    from concourse.bass import ts

    from concourse.masks import make_identity

    f32 = mybir.dt.float32
    bf16 = mybir.dt.bfloat16
    Alu = mybir.AluOpType
    Act = mybir.ActivationFunctionType

    nc = bacc.Bacc()
    hT_d = nc.declare_dram_parameter("hT", [F, N], bf16, isOutput=False)
    hTi_d = nc.declare_dram_parameter("hTi", [F, NI], bf16, isOutput=False)
    adjT_d = nc.declare_dram_parameter("adjT", [8 * 128, 4 * NI], bf16, isOutput=False)
    wr_d = nc.declare_dram_parameter("wr", [F, NH * O], bf16, isOutput=False)
    wTr_d = nc.declare_dram_parameter("wTr", [O, NH * F], bf16, isOutput=False)
    avec_d = nc.declare_dram_parameter("avec", [O, 2 * NH], bf16, isOutput=False)
    bias_d = nc.declare_dram_parameter("biasd", [1, O], f32, isOutput=False)
    out_d = nc.declare_dram_parameter("out", [NH, 128, NSUB * O], f32, isOutput=True)

    with tile.TileContext(nc) as tc:
        with (
            tc.tile_pool(name="sb", bufs=1) as sb,
            tc.tile_pool(name="sbr", bufs=2) as sbr,
            tc.tile_pool(name="sbo", bufs=3) as sbo,
            tc.tile_pool(name="pw", bufs=4, space="PSUM") as pw,
            tc.tile_pool(name="pacc", bufs=1, space="PSUM") as pacc,
        ):
            # ---- static SBUF tensors ----
            hT_sb = sb.tile([128, 2, N], bf16, name="hT_sb")
            hTi_sb = sb.tile([128, 2, NI], bf16, name="hTi_sb")
            adjT_sb = sb.tile([128, NCJ, NI], bf16, name="adjT_sb")
            wTr_sb = sb.tile([O, NH, F], bf16, name="wTr_sb")
            avec_sb = sb.tile([O, 2 * NH], bf16, name="avec_sb")
            bias_sb = sb.tile([1, O], f32, name="bias_sb")
            wall_sb = sb.tile([128, 2, NH * O + NH], bf16, name="wall_sb")
            vsrc_sb = sb.tile([128, 2, NH], bf16, name="vsrc_sb")
            ones_row = sb.tile([1, 128], bf16, name="ones_row")
            esrc3_rep = sb.tile([128, NH, NI], bf16, name="esrc3_rep")
            esrc3_row = sb.tile([1, NH, NI], bf16, name="esrc3_row")
            g_sb = sb.tile([128, NH, NCJ, O + 1], bf16, name="g_sb")
            edst3_sb = sb.tile([128, NCJ, NH], f32, name="edst3_sb")
            edst2_sb = sb.tile([128, NCJ, NH], f32, name="edst2_sb")
            bias_rep = sb.tile([128, O], f32, name="bias_rep")
            ostage = sb.tile([128, NH, NSUB, O], f32, name="ostage")
            negcb = sb.tile([128, 1], f32, name="negcb")
            nc.vector.memset(negcb[:, :], -CB)
            zerob = sb.tile([128, 1], f32, name="zerob")
            nc.vector.memset(zerob[:, :], 0.0)
            ident = sb.tile([128, 128], f32, name="ident")
            make_identity(nc, ident[:, :])

            # ---- DMA in ----  (small control tensors FIRST so the prologue
            # matmul chain can start while the bulk hT/adjT loads stream in)
            nc.sync.dma_start(
                hTi_sb, hTi_d[:, :].rearrange("(fc p) i -> p fc i", p=128)
            )
            wTr_r = wTr_d[:, :].rearrange("o (h f) -> o h f", h=NH)
            for h in range(NH):
                nc.sync.dma_start(wTr_sb[:, h, :], wTr_r[:, h, :])
            nc.sync.dma_start(avec_sb, avec_d[:, :])
            nc.sync.dma_start(bias_sb, bias_d[:, :])
            nc.sync.dma_start(
                wall_sb[:, :, 0 : NH * O],
                wr_d[:, :].rearrange("(fc p) m -> p fc m", p=128),
            )
            # adjT is host-pre-tiled as [8 groups, 128 partitions, 4*NI]:
            # each group DMA is 1 MiB with 8 KiB-contiguous runs per
            # partition (large-descriptor regime, ~340 GB/s)
            adjT_r = adjT_d[:, :].rearrange("(g p) x -> g p x", p=128)
            nc.sync.dma_start(
                adjT_sb[:, 0:4, :].rearrange("p c i -> p (c i)"), adjT_r[0]
            )
            hT_r = hT_d[:, :].rearrange("(fc p) j -> p fc j", p=128)
            nc.sync.dma_start(hT_sb[:, :, 0:512], hT_r[:, :, 0:512])
            nc.sync.dma_start(hT_sb[:, :, 512:N], hT_r[:, :, 512:N])
            for g in range(1, 7, 2):
                nc.sync.dma_start(
                    adjT_sb[:, 4 * g : 4 * g + 8, :].rearrange(
                        "p (g c) i -> p g (c i)", g=2
                    ),
                    adjT_r[g : g + 2].rearrange("g p x -> p g x"),
                )
            nc.sync.dma_start(
                adjT_sb[:, 28:32, :].rearrange("p c i -> p (c i)"), adjT_r[7]
            )

            nc.vector.memset(ones_row[:, :], 1.0)
            ones_f = sb.tile([1, 128], f32, name="ones_f")
            nc.vector.memset(ones_f[:, :], 1.0)

            # ---- v vectors: v[f] = sum_o wT[o,f] * a[o]  (cols: src, dst)
            for h in range(NH):
                for fc in range(2):
                    v_ps = pw.tile([128, 512], f32, name=f"v_ps_{h}_{fc}", tag="scratch")
                    nc.tensor.matmul(
                        v_ps[:, 0:2],
                        lhsT=wTr_sb[:, h, ts(fc, 128)],
                        rhs=avec_sb[:, 2 * h : 2 * h + 2],
                        start=True,
                        stop=True,
                    )
                    nc.scalar.copy(vsrc_sb[:, fc, h : h + 1], v_ps[:, 0:1])
                    nc.scalar.copy(
                        wall_sb[:, fc, NH * O + h : NH * O + h + 1], v_ps[:, 1:2]
                    )

            # ---- src row for this core's i-block, per head, then exp
            for h in range(NH):
                for seg in range(NSEG):
                    sr_ps = pw.tile([128, 512], f32, name=f"sr_ps_{h}_{seg}", tag="scratch")
                    for fc in range(2):
                        nc.tensor.matmul(
                            sr_ps[0:1, :],
                            lhsT=vsrc_sb[:, fc, h : h + 1],
                            rhs=hTi_sb[:, fc, ts(seg, 512)],
                            start=(fc == 0),
                            stop=(fc == 1),
                        )
                    nc.scalar.activation(
                        esrc3_row[:, h, ts(seg, 512)],
                        sr_ps[0:1, :],
                        Act.Exp,
                        scale=0.8,
                        bias=zerob[0:1, :],
                    )

            # ---- replicate e^{0.8 src} across partitions (K=1 ones matmul)
            for h in range(NH):
                for seg in range(NSEG):
                    rep_ps = pw.tile([128, 512], f32, name=f"rep_ps_{h}_{seg}", tag="scratch")
                    nc.tensor.matmul(
                        rep_ps[:, :],
                        lhsT=ones_row[:, :],
                        rhs=esrc3_row[:, h, ts(seg, 512)],
                        start=True,
                        stop=True,
                    )
                    nc.scalar.copy(esrc3_rep[:, h, ts(seg, 512)], rep_ps[:, :])


            # ---- bmm accumulators: psum [65, 512] per (head, i-segment)
            acc = [
                pacc.tile([O + 1, 512], f32, name=f"acc{g}", tag=f"acc{g}")
                for g in range(NH * NSEG)
            ]

            # ---- main loop over j-chunks, with the PE/ACT producer chain
            # (h_prime -> exp scalars -> G) running PIPE chunks ahead of the
            # DVE consumer so the vector engine never stalls on it.
            PIPE = 4

            def hp_block(c):
                # h_prime (2 heads) + dst (2 heads) in one accumulated matmul
                hp_ps = pw.tile([128, 512], f32, name=f"hp_ps_{c}", tag="scratch")[:, 0 : NH * O + NH]
                for fc in range(2):
                    nc.tensor.matmul(
                        hp_ps[:, :],
                        lhsT=hT_sb[:, fc, ts(c, 128)],
                        rhs=wall_sb[:, fc, :],
                        start=(fc == 0),
                        stop=(fc == 1),
                    )
                # exp the dst columns straight out of psum
                nc.scalar.activation(
                    edst3_sb[:, c, :],
                    hp_ps[:, NH * O : NH * O + NH],
                    Act.Exp,
                    scale=0.8,
                    bias=zerob[:, :],
                )
                nc.scalar.activation(
                    edst2_sb[:, c, :],
                    hp_ps[:, NH * O : NH * O + NH],
                    Act.Exp,
                    scale=0.2,
                    bias=negcb[:, :],
                )
                # G = e^{0.2 dst - CB} * [h_prime | 1] in bf16 (the edst2
                # row-factor rides the stationary operand; col O holds
                # edst2 itself, which makes the matmul's last row the
                # softmax denominator)
                for h in range(NH):
                    nc.scalar.activation(
                        g_sb[:, h, c, 0:O],
                        hp_ps[:, ts(h, O)],
                        Act.Copy,
                        scale=edst2_sb[:, c, h : h + 1],
                    )
                    nc.scalar.copy(g_sb[:, h, c, O : O + 1], edst2_sb[:, c, h : h + 1])

            for c in range(PIPE):
                hp_block(c)

            for c in range(NCJ):
                if c + PIPE < NCJ:
                    hp_block(c + PIPE)
                for h in range(NH):
                    r_t = sbr.tile([128, NI], bf16, name=f"r_{h}_{c}", tag=f"R{h}", bufs=4)
                    nc.vector.tensor_scalar(
                        out=r_t[:, :],
                        in0=esrc3_rep[:, h, :],
                        scalar1=edst3_sb[:, c, h : h + 1],
                        scalar2=1.0,
                        op0=Alu.mult,
                        op1=Alu.max,
                    )
                    p_t = sbr.tile([128, NI], bf16, name=f"p_{h}_{c}", tag=f"P{h}", bufs=6)
                    nc.vector.tensor_tensor(
                        out=p_t[:, :],
                        in0=r_t[:, :],
                        in1=adjT_sb[:, c, :],
                        op=Alu.mult,
                    )
                    for seg in range(NSEG):
                        nc.tensor.matmul(
                            acc[h * NSEG + seg][:, :],
                            lhsT=g_sb[:, h, c, :],
                            rhs=p_t[:, ts(seg, 512)],
                            start=(c == 0),
                            stop=(c == NCJ - 1),
                        )

            # ---- epilogue: transpose [65,512] -> [128,65], divide, bias,
            # stage per head, then one big DMA per head
            for h in range(NH):
                for seg in range(NSEG):
                    a_ps = acc[h * NSEG + seg]
                    tr_in = sbo.tile([O + 1, 512], f32, name=f"tr_{h}_{seg}", tag="trin")
                    nc.scalar.copy(tr_in[:, :], a_ps[:, :])
                    for q in range(4):
                        isub = seg * 4 + q
                        tr_ps = pw.tile([128, 512], f32, name=f"trp_{h}_{isub}", tag="scratch")
                        nc.tensor.transpose(
                            tr_ps[:, 0 : O + 1],
                            tr_in[:, ts(q, 128)],
                            ident[0 : O + 1, 0 : O + 1],
                        )
                        rec = sbr.tile([128, 1], f32, name=f"rec_{h}_{isub}", tag="rec")
                        nc.vector.reciprocal(rec[:, :], tr_ps[:, O : O + 1])
                        nc.scalar.activation(
                            ostage[:, h, isub, :],
                            tr_ps[:, 0:O],
                            Act.Copy,
                            scale=rec[:, :],
                        )
                nc.sync.dma_start(
                    out_d[h, :, :], ostage[:, h, :, :].rearrange("p s o -> p (s o)")
                )

    nc.finalize()
    return nc


def _prep_inputs(h, adj, w, a_src, a_dst, bias):
    """Host-side sharding / layout prep (no reference math)."""
    h = np.asarray(h, dtype=np.float32)
    adj = np.asarray(adj)
    w = np.asarray(w, dtype=np.float32)
    a_src = np.asarray(a_src, dtype=np.float32)
    a_dst = np.asarray(a_dst, dtype=np.float32)
    bias = np.asarray(bias, dtype=np.float32)

    hT = np.ascontiguousarray(h.T)                       # [F, N]
    adjT = np.ascontiguousarray(adj.T).astype(ml_dtypes.bfloat16)  # [N, N] 0/1

    in_maps = []
    for c in range(NCORES):
        hb, ib = c % 2, c // 2
        heads = [2 * hb, 2 * hb + 1]
        i0 = NI * ib
        w2 = w[heads]                                    # [2, F, O]
        wr = np.ascontiguousarray(w2.transpose(1, 0, 2).reshape(F, NH * O))
        wTr = np.ascontiguousarray(
            np.concatenate([w2[0].T, w2[1].T], axis=1)   # [O, 2F]
        )
        avec = np.ascontiguousarray(
            np.stack(
                [a_src[heads[0], :, 0], a_dst[heads[0], :, 0],
                 a_src[heads[1], :, 0], a_dst[heads[1], :, 0]],
                axis=1,
            )
        )                                                # [O, 4]
        in_maps.append(
            {
                "hT": hT.astype(ml_dtypes.bfloat16),
                "hTi": np.ascontiguousarray(hT[:, i0 : i0 + NI]).astype(
                    ml_dtypes.bfloat16
                ),
                "adjT": np.ascontiguousarray(
                    adjT[:, i0 : i0 + NI]
                    .reshape(8, 4, 128, NI)
                    .transpose(0, 2, 1, 3)
                    .reshape(8 * 128, 4 * NI)
                ),
                "wr": wr.astype(ml_dtypes.bfloat16),
                "wTr": wTr.astype(ml_dtypes.bfloat16),
                "avec": avec.astype(ml_dtypes.bfloat16),
                "biasd": bias.reshape(1, O),
            }
        )
    return in_maps


def kernel(h, adj, w, a_src, a_dst, bias):
    from concourse.bass_utils import run_bass_kernel_spmd

    if "nc" not in _CACHE:
        _CACHE["nc"] = _build()
    nc = _CACHE["nc"]

    in_maps = _prep_inputs(h, adj, w, a_src, a_dst, bias)
    res = run_bass_kernel_spmd(nc, in_maps, list(range(NCORES))).results

    out = np.empty((N, NHEAD, O), dtype=np.float32)
    for c in range(NCORES):
        hb, ib = c % 2, c // 2
        arr = res[c]["out"]  # [NH, 128, NSUB*O]
        for hh in range(NH):
            blk = (
                arr[hh]
                .reshape(128, NSUB, O)
                .transpose(1, 0, 2)
                .reshape(NI, O)
            )
            out[NI * ib : NI * (ib + 1), 2 * hb + hh, :] = blk
    out += np.asarray(bias, dtype=np.float32).reshape(1, 1, O)
    return out

